# Initial kernel scaffold
#
"""ExclusiveSelfAttention Trainium2 kernel (8-core SPMD, Megatron-style head TP).

Sharding: core c -> batch b = c // 4, head group g = c % 4 (heads 4g..4g+3).
Each core computes its 4 heads' attention plus the partial Wo projection for
its batch; the host sums the 4 partials per batch (the row-parallel
all-reduce, done at unshard time).

Device layout choices:
  - x is shipped pre-transposed (XT: [D, S]) so the contraction dim d sits on
    SBUF partitions for the QKV projections.
  - Q,K are produced directly transposed ([2*64 head-pair dims, S]) so the
    scores matmul needs no further transposes.
  - Scores are computed transposed (S^T[k, q]) so softmax-exp feeds the PV
    matmul directly; V carries an appended ones-column so PV row 64
    accumulates the softmax denominator for free.
  - exp skips max-subtraction (logits ~ N(0,1); exact same math in fp32).
  - The [65, 512] PV output is PE-transposed back to natural [q, 65] layout
    for the exclusive-projection correction, which then runs as a handful of
    batched DVE ops per head.

Walrus on TRN2 rejects instructions carrying too many semaphore waits
(Matmult: >1, others: >4 — "Too many sync wait commands"). _fixup_sync_waits
post-processes the scheduled BIR: excess waits are spilled onto NoOp
instructions inserted just before the offender on the same engine, which is
semantically identical (the waits still execute, in order, before it).
"""

import sys

if "/opt/trn_rl_repo" not in sys.path:
    sys.path.insert(0, "/opt/trn_rl_repo")

import numpy as np

NUM_HEADS = 16
HEAD_DIM = 64
D = NUM_HEADS * HEAD_DIM  # 1024
B = 2
S = 2048
EPS = 1e-8
SCALE = 1.0 / np.sqrt(HEAD_DIM)  # 0.125

N_CORES = 8
HPC = 4  # heads per core
CD = HPC * HEAD_DIM  # per-core slice of the model dim: 256
P = 128
DCH = D // P  # 8 d-chunks
SCH = S // P  # 16 sequence chunks
QB = 512  # query block
NQB = S // QB  # 4

_PROGRAM = None  # cached Bass program

# float32r streams fp32 bits through the PE at bf16 rate (1 cycle/column when
# the moving dim >= 256) vs plain fp32's 4 cycles/column, at slightly reduced
# multiply precision. All four matmul groups qualify (N in {256, 512}).
USE_FP32R = True


def _fixup_sync_waits(nc):
    """Spill semaphore waits beyond walrus's per-instruction limits onto
    NoOps inserted before the offending instruction (same engine)."""
    from concourse import mybir

    n_spill = 0
    for fn in nc.m.functions:
        for bb in fn.blocks:
            il = bb.instructions
            i = 0
            while i < len(il):
                ins = il[i]
                si = ins.sync_info
                if si is None or not si.on_wait:
                    i += 1
                    continue
                waits = list(si.on_wait)
                sem_idx = [
                    k for k, w in enumerate(waits)
                    if getattr(w, "sync_type", "semaphore") == "semaphore"
                ]
                if isinstance(ins, mybir.InstAllEngineBarrier):
                    i += 1
                    continue
                limit = 1  # this walrus allows one sem wait per instruction
                if len(waits) <= limit:
                    i += 1
                    continue
                n_excess = len(waits) - limit
                spill = [waits[k] for k in sem_idx[:n_excess]]
                if len(spill) < n_excess:
                    i += 1
                    continue  # non-semaphore waits; leave untouched
                keep = [w for w in waits if not any(w is s for s in spill)]
                pos = i
                for w in spill:  # one wait per NoOp — safest across opcodes
                    nop = mybir.InstNoOp(
                        name=f"I-wspill-{n_spill}",
                        text_hint="wait_spill",
                        bass_nofuse=True,
                    )
                    n_spill += 1
                    nop.engine = ins.engine
                    nop.sync_info = mybir.SyncInfo(on_wait=[w], on_update=[])
                    il.insert(pos, nop)
                    pos += 1
                    i += 1
                si.on_wait = keep
                i += 1
    return n_spill


def _build_program(fixup=True):
    import concourse.bass as bass
    import concourse.tile as tile
    from concourse import mybir
    from concourse.tile import add_dep_helper
    from contextlib import ExitStack

    f32 = mybir.dt.float32
    f32r = mybir.dt.float32r
    bf16 = mybir.dt.bfloat16
    AF = mybir.ActivationFunctionType

    nc = bass.Bass("TRN2", target_bir_lowering=False, debug=False)

    xt_d = nc.dram_tensor("xt", [D, S], f32, kind="ExternalInput").ap()
    wqt_d = nc.dram_tensor("wqt", [D, CD], f32, kind="ExternalInput").ap()
    wkt_d = nc.dram_tensor("wkt", [D, CD], f32, kind="ExternalInput").ap()
    wvt_d = nc.dram_tensor("wvt", [D, CD], f32, kind="ExternalInput").ap()
    wot_d = nc.dram_tensor("wot", [CD, D], f32, kind="ExternalInput").ap()
    gamma_d = nc.dram_tensor("gamma", [P, 1], f32, kind="ExternalInput").ap()
    out_d = nc.dram_tensor("out_p", [S, D], f32, kind="ExternalOutput").ap()

    mm_dt = f32r if USE_FP32R else f32

    with tile.TileContext(nc) as tc, ExitStack() as ctx:
        pers = ctx.enter_context(tc.tile_pool(name="pers", bufs=1))
        qk_pool = ctx.enter_context(tc.tile_pool(name="qk", bufs=1))
        e_pool = ctx.enter_context(tc.tile_pool(name="e", bufs=2))
        otu_pool = ctx.enter_context(tc.tile_pool(name="otu", bufs=2))
        o_pool = ctx.enter_context(tc.tile_pool(name="o", bufs=4))
        ot_pool = ctx.enter_context(tc.tile_pool(name="ot", bufs=2))
        small = ctx.enter_context(tc.tile_pool(name="small", bufs=2))
        tmp_pool = ctx.enter_context(tc.tile_pool(name="tmp", bufs=1))
        stage_pool = ctx.enter_context(tc.tile_pool(name="stage", bufs=2))
        ldraw_pool = ctx.enter_context(tc.tile_pool(name="ldraw", bufs=2))
        # PSUM budget (8 banks): s2 [128,1024] x2 = 4, po x2 = 2,
        # prj (filler projections) x1 = 1, pt (transposes) x1 = 1.
        psum_s = ctx.enter_context(tc.tile_pool(name="ps_s", bufs=2, space="PSUM"))
        psum_o = ctx.enter_context(tc.tile_pool(name="ps_o", bufs=2, space="PSUM"))
        psum_prj = ctx.enter_context(tc.tile_pool(name="ps_prj", bufs=1, space="PSUM"))
        psum_t = ctx.enter_context(tc.tile_pool(name="ps_t", bufs=1, space="PSUM"))

        ident = pers.tile([P, P], f32, tag="ident")
        nc.gpsimd.memset(ident, 0.0)
        nc.gpsimd.affine_select(
            out=ident,
            in_=ident,
            compare_op=mybir.AluOpType.not_equal,
            fill=1.0,
            base=0,
            pattern=[[-1, P]],
            channel_multiplier=1,
        )
        gamma = pers.tile([P, 1], f32, tag="gamma")
        nc.sync.dma_start(gamma, gamma_d)

        # DMA raw f32, then DVE-copy into f32r tiles (walrus requires fp32r
        # matmul operands to be produced rounded by a compute engine)
        _ld_state = {"n": 0}

        def load_f32r(dst, src_ap):
            # two half loads through half-size raw tiles, alternating between
            # the two HWDGE rings (SP and ACT queues) for parallel streams
            dma = None
            h = dst.shape[1] // 2
            for k in range(2):
                shape = [P, h] if len(dst.shape) == 2 else [P, h, dst.shape[2]]
                raw = ldraw_pool.tile(shape, f32, tag="ldraww", name="ldraw")
                eng = nc.sync if _ld_state["n"] % 2 == 0 else nc.scalar
                _ld_state["n"] += 1
                if len(dst.shape) == 2:
                    dma = eng.dma_start(raw, src_ap[:, k * h : (k + 1) * h])
                    nc.vector.tensor_copy(dst[:, k * h : (k + 1) * h], raw)
                else:
                    dma = eng.dma_start(raw, src_ap[:, k * h : (k + 1) * h, :])
                    nc.vector.tensor_copy(dst[:, k * h : (k + 1) * h, :], raw)
            return dma

        wq_sb = pers.tile([P, DCH, CD], mm_dt, tag="wq")
        wk_sb = pers.tile([P, DCH, CD], mm_dt, tag="wk")
        wv_sb = pers.tile([P, DCH, CD], mm_dt, tag="wv")
        wo_sb = pers.tile([P, CD // P, D], mm_dt, tag="wo")
        load_f32r(wv_sb, wvt_d.rearrange("(o p) e -> p o e", p=P))
        xt_sb = []
        xt_dmas = []
        for d in range(DCH):
            t = pers.tile([P, S], mm_dt, tag=f"xt{d}", name=f"xt_sb{d}")
            xt_dmas.append(load_f32r(t, xt_d[d * P : (d + 1) * P, :]))
            xt_sb.append(t)
        load_f32r(wq_sb, wqt_d.rearrange("(o p) e -> p o e", p=P))
        load_f32r(wk_sb, wkt_d.rearrange("(o p) e -> p o e", p=P))
        load_f32r(wo_sb, wot_d.rearrange("(o p) f -> p o f", p=P))

        # preload the ACT exp table while DMAs run
        exp_warm = stage_pool.tile([P, P], f32, tag="expwarm")
        nc.scalar.activation(exp_warm, ident, AF.Exp, scale=0.01)

        # HAM warmup: a short dense matmul burst gated on the 6th x-chunk
        # DMA, so the PE array is at full clock when projections start
        for w in range(8):
            wm = psum_s.tile([P, P], f32, tag="s2")
            mi = nc.tensor.matmul(wm, lhsT=ident, rhs=ident, start=True, stop=True)
            if w == 0:
                add_dep_helper(mi.ins, xt_dmas[5].ins, reason="warmup gate")

        # V in natural layout [k-chunk, head, 65], col 64 = ones (denominator trick)
        v_sb = pers.tile([P, SCH, HPC, HEAD_DIM + 1], mm_dt, tag="v")
        ones_col = pers.tile([P, 1], f32, tag="ones")
        nc.vector.memset(ones_col, 1.0)
        nc.vector.tensor_copy(
            out=v_sb[:, :, :, HEAD_DIM : HEAD_DIM + 1],
            in_=ones_col[:, None, :, None].to_broadcast([P, SCH, HPC, 1]),
        )
        for c in range(SCH):
            pv = psum_s.tile([P, CD], f32, tag="s2")
            for d in range(DCH):
                nc.tensor.matmul(
                    pv,
                    lhsT=xt_sb[d][:, c * P : (c + 1) * P],
                    rhs=wv_sb[:, d, :],
                    start=(d == 0),
                    stop=(d == DCH - 1),
                )
            nc.vector.tensor_copy(
                out=v_sb[:, c, :, 0:HEAD_DIM],
                in_=pv.rearrange("p (h e) -> p h e", h=HPC),
            )

        ot_tiles = []
        o_tiles = {}  # hg -> o_h tile

        def project_block(wsb, dst, pair, blk):
            """Thunks for one [128,512] projection block: 8 accumulating
            matmuls into the dedicated prj PSUM bank + the copy-out."""
            state = {}

            def mk_mm(d):
                def run():
                    if d == 0:
                        state["pq"] = psum_prj.tile([P, QB], f32, tag="prj", name="pq_prj")
                    nc.tensor.matmul(
                        state["pq"],
                        lhsT=wsb[:, d, pair * P : (pair + 1) * P],
                        rhs=xt_sb[d][:, blk * QB : (blk + 1) * QB],
                        start=(d == 0),
                        stop=(d == DCH - 1),
                        skip_group_check=True,
                    )
                return run

            thunks = [mk_mm(d) for d in range(DCH)]

            def cp():
                nc.vector.tensor_copy(
                    out=dst[:, blk * QB : (blk + 1) * QB], in_=state["pq"]
                )

            thunks.append(cp)
            return thunks

        def project_pair_now(pair, qt, kt):
            for wsb, dst in ((wq_sb, qt), (wk_sb, kt)):
                for blk in range(NQB):
                    for th in project_block(wsb, dst, pair, blk):
                        th()

        from collections import deque

        filler_q = deque()

        def pop_filler(n=1):
            for _ in range(n):
                if not filler_q:
                    return
                filler_q.popleft()()

        def head_setup(hg):
            """Per-head vns/rvns (depends only on V): gamma/vns folded."""
            v_view = v_sb[:, :, hg, 0:HEAD_DIM].bitcast(f32)
            tmp = tmp_pool.tile([P, SCH, HEAD_DIM], f32, tag="tmp")
            nc.vector.tensor_mul(tmp, v_view, v_view)
            vns = small.tile([P, SCH, 1], f32, tag=f"vns{hg % 2}", name="vns")
            nc.vector.reduce_sum(vns, tmp, axis=mybir.AxisListType.X)
            nc.vector.tensor_scalar_add(vns, vns, float(EPS))
            rvns = small.tile([P, SCH, 1], f32, tag=f"rvns{hg % 2}", name="rvns")
            nc.vector.reciprocal(rvns, vns)
            nc.vector.tensor_scalar_mul(rvns, rvns, gamma)
            return rvns

        def correct_j(hg, j, rvns):
            """Correction for q-chunks 4j..4j+3 of head hg (runs on DVE
            while later q-blocks are still in the matmul pipeline)."""
            o_h = o_tiles[hg]
            cs = slice(4 * j, 4 * j + 4)
            v_view = v_sb[:, cs, hg, 0:HEAD_DIM].bitcast(f32)
            ou = o_h[:, cs, 0:HEAD_DIM]
            den = o_h[:, cs, HEAD_DIM : HEAD_DIM + 1]
            rden = small.tile([P, 4, 1], f32, tag="rden")
            nc.vector.reciprocal(rden, den)
            tmp = tmp_pool.tile([P, 4, HEAD_DIM], f32, tag="tmpj")
            nc.vector.tensor_mul(tmp, ou, v_view)
            dotu = small.tile([P, 4, 1], f32, tag="dotu")
            nc.vector.reduce_sum(dotu, tmp, axis=mybir.AxisListType.X)
            cu = small.tile([P, 4, 1], f32, tag="cu")
            nc.vector.tensor_mul(cu, dotu, rvns[:, cs])
            nc.vector.tensor_mul(tmp, v_view, cu.to_broadcast([P, 4, HEAD_DIM]))
            nc.vector.tensor_sub(ou, ou, tmp)
            nc.vector.tensor_mul(ou, ou, rden.to_broadcast([P, 4, HEAD_DIM]))

        def ftr_j(pair, ot, j):
            """Transpose corrected O chunks 4j..4j+3 back into OT rows —
            returned as filler thunks."""
            thunks = []
            for c in range(4 * j, 4 * j + 4):
                for h in range(2):
                    def th(c=c, h=h):
                        o_h = o_tiles[pair * 2 + h]
                        lo = h * HEAD_DIM
                        pt2 = psum_t.tile(
                            [P, 4 * (HEAD_DIM + 1)], f32, tag="pt", name="pt2"
                        )
                        nc.tensor.transpose(
                            pt2[0:HEAD_DIM, 0:P], o_h[:, c, 0:HEAD_DIM], ident
                        )
                        nc.vector.tensor_copy(
                            out=ot[lo : lo + HEAD_DIM, c * P : (c + 1) * P],
                            in_=pt2[0:HEAD_DIM, 0:P],
                        )
                    thunks.append(th)
            return thunks

        def attention_pair(pair, qt, kt, ot):
            """Both heads together: the two K=64 score matmuls use disjoint
            PE row groups (base partitions 0/64) and run concurrently.
            Fillers (next pair's projections, pending output transposes) are
            drip-fed one per chunk to absorb exp-wait stalls and keep the PE
            clock warm. Correction runs per-q-block on the DVE as soon as
            each block's PV output lands."""
            o_hA = o_pool.tile([P, SCH, HEAD_DIM + 1], f32, tag="oh")
            o_hB = o_pool.tile([P, SCH, HEAD_DIM + 1], f32, tag="oh")
            o_tiles[pair * 2] = o_hA
            o_tiles[pair * 2 + 1] = o_hB
            rvA = head_setup(pair * 2)
            rvB = head_setup(pair * 2 + 1)
            for j in range(NQB):
                po_A = psum_o.tile([HEAD_DIM + 1, QB], f32, tag="po")
                po_B = psum_o.tile([HEAD_DIM + 1, QB], f32, tag="po")

                def scores(c):
                    s2 = psum_s.tile([P, 2 * QB], f32, tag="s2")
                    for h, lo in ((0, 0), (1, HEAD_DIM)):
                        nc.tensor.matmul(
                            s2[:, h * QB : (h + 1) * QB],
                            lhsT=kt[lo : lo + HEAD_DIM, c * P : (c + 1) * P],
                            rhs=qt[lo : lo + HEAD_DIM, j * QB : (j + 1) * QB],
                            start=True,
                            stop=True,
                        )
                    e2 = e_pool.tile([P, 2 * QB], mm_dt, tag="e2")
                    nc.scalar.activation(e2, s2, AF.Exp, scale=float(SCALE))
                    return e2

                def pv(c, e2):
                    for po, h in ((po_A, 0), (po_B, 1)):
                        nc.tensor.matmul(
                            po,
                            lhsT=v_sb[:, c, pair * 2 + h, :],
                            rhs=e2[:, h * QB : (h + 1) * QB],
                            start=(c == 0),
                            stop=(c == SCH - 1),
                            skip_group_check=True,
                        )

                prev_e2 = None
                for c in range(SCH):
                    e2 = scores(c)
                    if prev_e2 is not None:
                        pv(c - 1, prev_e2)
                    pop_filler(2 if len(filler_q) > 48 else 1)
                    prev_e2 = e2
                pv(SCH - 1, prev_e2)

                for po, o_h in ((po_A, o_hA), (po_B, o_hB)):
                    otu = otu_pool.tile([HEAD_DIM + 1, QB], f32, tag="otu")
                    nc.vector.tensor_copy(otu, po)
                    pt = psum_t.tile([P, 4 * (HEAD_DIM + 1)], f32, tag="pt")
                    for i in range(4):
                        nc.tensor.transpose(
                            pt[:, i * 65 : (i + 1) * 65],
                            otu[:, i * P : (i + 1) * P],
                            ident[0:65, 0:65],
                        )
                    nc.vector.tensor_copy(
                        out=o_h[:, j * 4 : (j + 1) * 4, :],
                        in_=pt.rearrange("p (i e) -> p i e", i=4),
                    )
                correct_j(pair * 2, j, rvA)
                correct_j(pair * 2 + 1, j, rvB)
                filler_q.extend(ftr_j(pair, ot, j))

        ot0 = ot_pool.tile([P, S], mm_dt, tag="ot")
        ot1 = ot_pool.tile([P, S], mm_dt, tag="ot")
        ot_tiles = [ot0, ot1]
        qt0 = qk_pool.tile([P, S], mm_dt, tag="qt")
        kt0 = qk_pool.tile([P, S], mm_dt, tag="kt")
        qt1 = qk_pool.tile([P, S], mm_dt, tag="qt1")
        kt1 = qk_pool.tile([P, S], mm_dt, tag="kt1")

        # pair-0 projections run dense right after V
        project_pair_now(0, qt0, kt0)

        # seed fillers: pair-1 projections (K first, then Q blocks in the
        # order attention will need them)
        for blk in range(NQB):
            filler_q.extend(project_block(wk_sb, kt1, 1, blk))
        for blk in range(NQB):
            filler_q.extend(project_block(wq_sb, qt1, 1, blk))

        attention_pair(0, qt0, kt0, ot0)
        attention_pair(1, qt1, kt1, ot1)

        # tail: flush pending transposes, then the partial output
        # projection out_p[s, f] = sum_c OT[c, s] * WoT[c, f]
        while filler_q:
            filler_q.popleft()()
        for sc in range(SCH):
            for fb in range(2):
                pp = psum_s.tile([P, QB], f32, tag="s2")
                for pair in range(2):
                    nc.tensor.matmul(
                        pp,
                        lhsT=ot_tiles[pair][:, sc * P : (sc + 1) * P],
                        rhs=wo_sb[:, pair, fb * QB : (fb + 1) * QB],
                        start=(pair == 0),
                        stop=(pair == 1),
                    )
                stage = stage_pool.tile([P, QB], f32, tag="stage")
                nc.any.tensor_copy(stage, pp)
                nc.sync.dma_start(
                    out_d[sc * P : (sc + 1) * P, fb * QB : (fb + 1) * QB], stage
                )

    if fixup:
        _fixup_sync_waits(nc)
    return nc


def _get_program():
    global _PROGRAM
    if _PROGRAM is None:
        _PROGRAM = _build_program()
    return _PROGRAM


def _make_in_maps(x, Wq, Wk, Wv, Wo, xsa_scale):
    x = np.ascontiguousarray(np.asarray(x, dtype=np.float32))
    Wq = np.ascontiguousarray(np.asarray(Wq, dtype=np.float32))
    Wk = np.ascontiguousarray(np.asarray(Wk, dtype=np.float32))
    Wv = np.ascontiguousarray(np.asarray(Wv, dtype=np.float32))
    Wo = np.ascontiguousarray(np.asarray(Wo, dtype=np.float32))
    gamma = np.full((P, 1), np.float32(np.asarray(xsa_scale).reshape(-1)[0]))

    in_maps = []
    for core in range(N_CORES):
        b = core // 4
        g = core % 4
        cs = slice(g * CD, (g + 1) * CD)
        in_maps.append(
            {
                "xt": np.ascontiguousarray(x[b].T),
                "wqt": np.ascontiguousarray(Wq[cs, :].T),
                "wkt": np.ascontiguousarray(Wk[cs, :].T),
                "wvt": np.ascontiguousarray(Wv[cs, :].T),
                "wot": np.ascontiguousarray(Wo[:, cs].T),
                "gamma": gamma,
            }
        )
    return in_maps


def _ensure_ntff_hook():
    """The agent image's antenv lacks axon_hooks; reconstruct it so
    run_bass_kernel_spmd(trace=True) can capture NTFF profiles."""
    import sys as _sys
    import types

    if "antenv.axon_hooks" in _sys.modules:
        return
    mod = types.ModuleType("antenv.axon_hooks")
    state = {"hook": None}
    mod.set_axon_ntff_profile_hook = lambda h: state.__setitem__("hook", h)
    mod.get_axon_ntff_profile_hook = lambda: state["hook"]
    _sys.modules["antenv.axon_hooks"] = mod
    try:
        import antenv

        antenv.axon_hooks = mod
    except ImportError:
        pass
    try:
        from trn_agent_boot.trn_boot import _ntff_profile_via_ctypes

        mod.set_axon_ntff_profile_hook(
            _ntff_profile_via_ctypes("/opt/axon/libaxon_pjrt.so")
        )
    except Exception as e:  # degrade to no-trace
        print(f"ntff hook setup failed ({e}); tracing disabled", file=sys.stderr)


def _run(inputs, trace=False, **kwargs):
    from concourse.bass_utils import run_bass_kernel_spmd

    if trace:
        _ensure_ntff_hook()

    nc = _get_program()
    in_maps = _make_in_maps(
        inputs["x"], inputs["Wq"], inputs["Wk"], inputs["Wv"], inputs["Wo"],
        inputs["xsa_scale"],
    )
    res = run_bass_kernel_spmd(
        nc, in_maps, core_ids=list(range(N_CORES)), trace=trace, **kwargs
    )
    parts = [np.asarray(res.results[c]["out_p"]) for c in range(N_CORES)]
    out = np.stack(
        [
            parts[0] + parts[1] + parts[2] + parts[3],
            parts[4] + parts[5] + parts[6] + parts[7],
        ]
    ).astype(np.float32)
    return out, res


def kernel(**inputs) -> np.ndarray:
    out, _ = _run(inputs, trace=False)
    return out



# revision 20
# speedup vs baseline: 1.3180x; 1.3180x over previous
"""ExclusiveSelfAttention Trainium2 kernel (8-core SPMD, Megatron-style head TP).

Sharding: core c -> batch b = c // 4, head group g = c % 4 (heads 4g..4g+3).
Each core computes its 4 heads' attention plus the partial Wo projection for
its batch; the host sums the 4 partials per batch (the row-parallel
all-reduce, done at unshard time).

Device layout choices:
  - x is shipped pre-transposed (XT: [D, S]) so the contraction dim d sits on
    SBUF partitions for the QKV projections.
  - Q,K are produced directly transposed ([2*64 head-pair dims, S]) so the
    scores matmul needs no further transposes.
  - Scores are computed transposed (S^T[k, q]) so softmax-exp feeds the PV
    matmul directly; V carries an appended ones-column so PV row 64
    accumulates the softmax denominator for free.
  - exp skips max-subtraction (logits ~ N(0,1); exact same math in fp32).
  - The [65, 512] PV output is PE-transposed back to natural [q, 65] layout
    for the exclusive-projection correction, which then runs as a handful of
    batched DVE ops per head.

Scheduling (v2): the kernel streams from the first microsecond instead of
serializing load -> project -> attend -> project-out:
  - x arrives in [128, 512] column pieces; V-chunks / K / Q projection blocks
    are emitted as soon as their pieces land.
  - attention pair 0 starts after just {K0 block 0, V chunks 0-3, Q0 block 0};
    the remaining V/K0/Q0 work plus all of pair-1's projections drain through
    the filler queue inside the attention c-loop. Explicit drain-to-boundary
    bookkeeping guarantees every filler that writes a region is emitted
    before the instruction that reads it.
  - the output projection runs incrementally: as soon as pair-1's j-block is
    corrected and transposed, its four OT column chunks are projected through
    Wo and DMA'd out as fillers, so the kernel has no cold tail (the HAM
    down-clocks the core to half speed when PE activity drops; the v1 tail
    ran ~46us at half clock).

Walrus on TRN2 rejects instructions carrying too many semaphore waits
(Matmult: >1, others: >4 — "Too many sync wait commands"). _fixup_sync_waits
post-processes the scheduled BIR: excess waits are spilled onto NoOp
instructions inserted just before the offender on the same engine, which is
semantically identical (the waits still execute, in order, before it).
"""

import sys

if "/opt/trn_rl_repo" not in sys.path:
    sys.path.insert(0, "/opt/trn_rl_repo")

import numpy as np

NUM_HEADS = 16
HEAD_DIM = 64
D = NUM_HEADS * HEAD_DIM  # 1024
B = 2
S = 2048
EPS = 1e-8
SCALE = 1.0 / np.sqrt(HEAD_DIM)  # 0.125

N_CORES = 8
HPC = 4  # heads per core
CD = HPC * HEAD_DIM  # per-core slice of the model dim: 256
P = 128
DCH = D // P  # 8 d-chunks
SCH = S // P  # 16 sequence chunks
QB = 512  # query block
NQB = S // QB  # 4

_PROGRAM = None  # cached Bass program

# fp16 streams through the PE at 1 cycle/column (like bf16) with 10 mantissa
# bits — simulated end-to-end precision 7.4e-4 scale-relative vs the f64
# reference (fp32r baseline: 3.4e-4; bf16: 6.5e-3). fp16 operands need no
# f32r-style "produced by a compute engine" rounding, so inputs DMA straight
# into their tiles with no bounce buffers or casts, and the 2-byte streams
# halve DMA bytes and PE input energy (headroom for the HAM power governor).
USE_FP16 = True


def _fixup_sync_waits(nc):
    """Spill semaphore waits beyond walrus's per-instruction limits onto
    NoOps inserted before the offending instruction (same engine)."""
    from concourse import mybir

    n_spill = 0
    for fn in nc.m.functions:
        for bb in fn.blocks:
            il = bb.instructions
            i = 0
            while i < len(il):
                ins = il[i]
                si = ins.sync_info
                if si is None or not si.on_wait:
                    i += 1
                    continue
                waits = list(si.on_wait)
                sem_idx = [
                    k for k, w in enumerate(waits)
                    if getattr(w, "sync_type", "semaphore") == "semaphore"
                ]
                if isinstance(ins, mybir.InstAllEngineBarrier):
                    i += 1
                    continue
                limit = 1  # this walrus allows one sem wait per instruction
                if len(waits) <= limit:
                    i += 1
                    continue
                n_excess = len(waits) - limit
                spill = [waits[k] for k in sem_idx[:n_excess]]
                if len(spill) < n_excess:
                    i += 1
                    continue  # non-semaphore waits; leave untouched
                keep = [w for w in waits if not any(w is s for s in spill)]
                pos = i
                for w in spill:  # one wait per NoOp — safest across opcodes
                    nop = mybir.InstNoOp(
                        name=f"I-wspill-{n_spill}",
                        text_hint="wait_spill",
                        bass_nofuse=True,
                    )
                    n_spill += 1
                    nop.engine = ins.engine
                    nop.sync_info = mybir.SyncInfo(on_wait=[w], on_update=[])
                    il.insert(pos, nop)
                    pos += 1
                    i += 1
                si.on_wait = keep
                i += 1
    return n_spill


def _build_program(fixup=True):
    import concourse.bass as bass
    import concourse.tile as tile
    from concourse import mybir
    from concourse.tile import add_dep_helper
    from contextlib import ExitStack
    from collections import deque

    f32 = mybir.dt.float32
    f16 = mybir.dt.float16
    AF = mybir.ActivationFunctionType

    nc = bass.Bass("TRN2", target_bir_lowering=False, debug=False)

    in_dt = mybir.dt.float16 if USE_FP16 else mybir.dt.float32
    xt_d = nc.dram_tensor("xt", [D, S], in_dt, kind="ExternalInput").ap()
    wqt_d = nc.dram_tensor("wqt", [D, CD], in_dt, kind="ExternalInput").ap()
    wkt_d = nc.dram_tensor("wkt", [D, CD], in_dt, kind="ExternalInput").ap()
    wvt_d = nc.dram_tensor("wvt", [D, CD], in_dt, kind="ExternalInput").ap()
    wot_d = nc.dram_tensor("wot", [CD, D], in_dt, kind="ExternalInput").ap()
    gamma_d = nc.dram_tensor("gamma", [P, 1], f32, kind="ExternalInput").ap()
    out_d = nc.dram_tensor("out_p", [S, D], f32, kind="ExternalOutput").ap()

    mm_dt = f16 if USE_FP16 else f32

    with tile.TileContext(nc) as tc, ExitStack() as ctx:
        pers = ctx.enter_context(tc.tile_pool(name="pers", bufs=1))
        qk_pool = ctx.enter_context(tc.tile_pool(name="qk", bufs=1))
        e_pool = ctx.enter_context(tc.tile_pool(name="e", bufs=3))
        otu_pool = ctx.enter_context(tc.tile_pool(name="otu", bufs=2))
        o_pool = ctx.enter_context(tc.tile_pool(name="o", bufs=4))
        ot_pool = ctx.enter_context(tc.tile_pool(name="ot", bufs=2))
        small = ctx.enter_context(tc.tile_pool(name="small", bufs=2))
        tmp_pool = ctx.enter_context(tc.tile_pool(name="tmp", bufs=1))
        stage_pool = ctx.enter_context(tc.tile_pool(name="stage", bufs=3))
        # PSUM budget (8 banks): s2 [128,1024] x2 = 4, po x2 = 2,
        # prj (projection / output-projection fillers) = 1, pt (transposes) = 1.
        psum_s = ctx.enter_context(tc.tile_pool(name="ps_s", bufs=2, space="PSUM"))
        psum_o = ctx.enter_context(tc.tile_pool(name="ps_o", bufs=2, space="PSUM"))
        psum_prj = ctx.enter_context(tc.tile_pool(name="ps_prj", bufs=1, space="PSUM"))
        psum_t = ctx.enter_context(tc.tile_pool(name="ps_t", bufs=1, space="PSUM"))

        ident = pers.tile([P, P], f32, tag="ident")
        nc.gpsimd.memset(ident, 0.0)
        nc.gpsimd.affine_select(
            out=ident,
            in_=ident,
            compare_op=mybir.AluOpType.not_equal,
            fill=1.0,
            base=0,
            pattern=[[-1, P]],
            channel_multiplier=1,
        )
        gamma = pers.tile([P, 1], f32, tag="gamma")
        nc.sync.dma_start(gamma, gamma_d)

        # ---------------- input streaming ----------------
        # fp16 operands have no walrus "rounded by a compute engine" rule
        # (that is fp32r-only), so inputs DMA straight into their final
        # tiles: no bounce buffers, no casts, nothing to ping-pong against.
        dma_state = {"n": 0}

        def dma_eng():
            eng = nc.sync if dma_state["n"] % 2 == 0 else nc.scalar
            dma_state["n"] += 1
            return eng

        def load_direct(dst_slice, src_ap):
            return dma_eng().dma_start(dst_slice, src_ap)

        wq_sb = pers.tile([P, DCH, CD], mm_dt, tag="wq")
        wk_sb = pers.tile([P, DCH, CD], mm_dt, tag="wk")
        wv_sb = pers.tile([P, DCH, CD], mm_dt, tag="wv")
        wo_sb = pers.tile([P, CD // P, D], mm_dt, tag="wo")
        xt_sb = [
            pers.tile([P, S], mm_dt, tag=f"xt{d}", name=f"xt_sb{d}")
            for d in range(DCH)
        ]
        x_dma = {}

        def load_x_half(d, half):
            x_dma[(d, half)] = load_direct(
                xt_sb[d][:, half * (S // 2) : (half + 1) * (S // 2)],
                xt_d[d * P : (d + 1) * P, half * (S // 2) : (half + 1) * (S // 2)],
            )

        # arrival order = need order: wv/wk (V + K0 first), x first halves,
        # wq, x second halves, wo (only needed by the output projection).
        load_direct(wv_sb, wvt_d.rearrange("(o p) e -> p o e", p=P))
        load_direct(wk_sb, wkt_d.rearrange("(o p) e -> p o e", p=P))
        for d in range(DCH):
            load_x_half(d, 0)
        load_direct(wq_sb, wqt_d.rearrange("(o p) e -> p o e", p=P))
        for d in range(DCH):
            load_x_half(d, 1)
        load_direct(wo_sb, wot_d.rearrange("(o p) f -> p o f", p=P))

        # preload the ACT exp table while DMAs run
        exp_warm = stage_pool.tile([P, P], f32, tag="expwarm")
        nc.scalar.activation(exp_warm, ident, AF.Exp, scale=0.01)

        # HAM warmup: a short dense matmul burst gated on an early x piece,
        # so the PE array is at speed when the first projections start
        for w in range(8):
            wm = psum_s.tile([P, P], f32, tag="s2")
            mi = nc.tensor.matmul(wm, lhsT=ident, rhs=ident, start=True, stop=True)
            if w == 0:
                add_dep_helper(mi.ins, x_dma[(2, 0)].ins, reason="warmup gate")

        # V in natural layout [k-chunk, head, 65], col 64 = ones (denominator trick)
        v_sb = pers.tile([P, SCH, HPC, HEAD_DIM + 1], mm_dt, tag="v")
        ones_col = pers.tile([P, 1], f32, tag="ones")
        nc.vector.memset(ones_col, 1.0)
        nc.vector.tensor_copy(
            out=v_sb[:, :, :, HEAD_DIM : HEAD_DIM + 1],
            in_=ones_col[:, None, :, None].to_broadcast([P, SCH, HPC, 1]),
        )

        # ---------------- filler queue with drain boundaries ----------------
        filler_q = deque()
        outstanding = {}

        def enqueue_unit(uid, thunks):
            if uid is not None:
                outstanding[uid] = outstanding.get(uid, 0) + len(thunks)
            for th in thunks:
                filler_q.append((uid, th))

        def pop_one():
            if not filler_q:
                return False
            uid, th = filler_q.popleft()
            th()
            if uid is not None:
                outstanding[uid] -= 1
            return True

        def pop_filler(n=1):
            for _ in range(n):
                if not pop_one():
                    return

        def drain(uid):
            while outstanding.get(uid, 0) > 0:
                if not pop_one():
                    raise RuntimeError(f"drain {uid}: queue underflow")

        def v_chunk_inline(c):
            """One V chunk: 8 accumulating matmuls + natural-layout copy.
            Runs in the double-buffered s2 ring (preamble only)."""
            pv = psum_s.tile([P, CD], f32, tag="s2", name="pv")
            for d in range(DCH):
                nc.tensor.matmul(
                    pv,
                    lhsT=xt_sb[d][:, c * P : (c + 1) * P],
                    rhs=wv_sb[:, d, :],
                    start=(d == 0),
                    stop=(d == DCH - 1),
                )
            nc.vector.tensor_copy(
                out=v_sb[:, c, :, 0:HEAD_DIM],
                in_=pv.rearrange("p (h e) -> p h e", h=HPC),
            )

        def v_chunk_thunks(c):
            """Filler-queue variant of a V chunk, through the shared prj
            bank (popped inside attention pair 0's c-loop)."""
            state = {}

            def mk(d):
                def run():
                    if d == 0:
                        state["pv"] = psum_prj.tile([P, CD], f32, tag="prj", name="pv")
                    nc.tensor.matmul(
                        state["pv"],
                        lhsT=xt_sb[d][:, c * P : (c + 1) * P],
                        rhs=wv_sb[:, d, :],
                        start=(d == 0),
                        stop=(d == DCH - 1),
                        skip_group_check=True,
                    )
                return run

            thunks = [mk(d) for d in range(DCH)]

            def cp():
                nc.vector.tensor_copy(
                    out=v_sb[:, c, :, 0:HEAD_DIM],
                    in_=state["pv"].rearrange("p (h e) -> p h e", h=HPC),
                )

            thunks.append(cp)
            return thunks

        def project_block(wsb, dst, pair, blk, pool=None):
            """Thunks for one [128,512] projection block: 8 accumulating
            matmuls into a PSUM bank + the copy-out. Fillers use the
            dedicated prj bank; the inline preamble passes psum_s."""
            state = {}
            pool = pool or psum_prj
            tag = "s2" if pool is psum_s else "prj"

            def mk_mm(d):
                def run():
                    if d == 0:
                        state["pq"] = pool.tile([P, QB], f32, tag=tag, name="pq_prj")
                    nc.tensor.matmul(
                        state["pq"],
                        lhsT=wsb[:, d, pair * P : (pair + 1) * P],
                        rhs=xt_sb[d][:, blk * QB : (blk + 1) * QB],
                        start=(d == 0),
                        stop=(d == DCH - 1),
                        skip_group_check=True,
                    )
                return run

            thunks = [mk_mm(d) for d in range(DCH)]

            def cp():
                nc.vector.tensor_copy(
                    out=dst[:, blk * QB : (blk + 1) * QB], in_=state["pq"]
                )

            thunks.append(cp)
            return thunks

        ot0 = ot_pool.tile([P, S], mm_dt, tag="ot")
        ot1 = ot_pool.tile([P, S], mm_dt, tag="ot")
        ot_tiles = [ot0, ot1]
        qt0 = qk_pool.tile([P, S], mm_dt, tag="qt")
        kt0 = qk_pool.tile([P, S], mm_dt, tag="kt")
        qt1 = qk_pool.tile([P, S], mm_dt, tag="qt1")
        kt1 = qk_pool.tile([P, S], mm_dt, tag="kt1")
        qk_t = {(0, "q"): qt0, (0, "k"): kt0, (1, "q"): qt1, (1, "k"): kt1}

        def outproj_unit(sc):
            """Partial output projection + DMA for OT column chunk sc:
            out_p[s, f] = sum_c OT[c, s] * WoT[c, f]."""
            ths = []
            for fb in range(2):
                st = {}

                def m0(sc=sc, fb=fb, st=st):
                    st["pp"] = psum_prj.tile([P, QB], f32, tag="prj", name="pp")
                    nc.tensor.matmul(
                        st["pp"],
                        lhsT=ot0[:, sc * P : (sc + 1) * P],
                        rhs=wo_sb[:, 0, fb * QB : (fb + 1) * QB],
                        start=True,
                        stop=False,
                        skip_group_check=True,
                    )

                def m1(sc=sc, fb=fb, st=st):
                    nc.tensor.matmul(
                        st["pp"],
                        lhsT=ot1[:, sc * P : (sc + 1) * P],
                        rhs=wo_sb[:, 1, fb * QB : (fb + 1) * QB],
                        start=False,
                        stop=True,
                        skip_group_check=True,
                    )

                def cp(sc=sc, fb=fb, st=st):
                    stg = stage_pool.tile([P, QB], f32, tag="stage")
                    nc.vector.tensor_copy(stg, st["pp"])
                    nc.sync.dma_start(
                        out_d[sc * P : (sc + 1) * P, fb * QB : (fb + 1) * QB], stg
                    )

                ths += [m0, m1, cp]
            return ths

        o_tiles = {}  # hg -> o_h tile

        def head_setup(hg):
            """Per-head vns/rvns (depends only on V): gamma/vns folded."""
            v_view = v_sb[:, :, hg, 0:HEAD_DIM]
            tmp = tmp_pool.tile([P, SCH, HEAD_DIM], f32, tag="tmp")
            nc.vector.tensor_mul(tmp, v_view, v_view)
            vns = small.tile([P, SCH, 1], f32, tag=f"vns{hg % 2}", name="vns")
            nc.vector.reduce_sum(vns, tmp, axis=mybir.AxisListType.X)
            nc.vector.tensor_scalar_add(vns, vns, float(EPS))
            rvns = small.tile([P, SCH, 1], f32, tag=f"rvns{hg % 2}", name="rvns")
            nc.vector.reciprocal(rvns, vns)
            nc.vector.tensor_scalar_mul(rvns, rvns, gamma)
            return rvns

        def correct_j(hg, j, rvns):
            """Correction for q-chunks 4j..4j+3 of head hg (runs on DVE
            while later q-blocks are still in the matmul pipeline)."""
            o_h = o_tiles[hg]
            cs = slice(4 * j, 4 * j + 4)
            v_view = v_sb[:, cs, hg, 0:HEAD_DIM]
            ou = o_h[:, cs, 0:HEAD_DIM]
            den = o_h[:, cs, HEAD_DIM : HEAD_DIM + 1]
            rden = small.tile([P, 4, 1], f32, tag="rden")
            nc.vector.reciprocal(rden, den)
            tmp = tmp_pool.tile([P, 4, HEAD_DIM], f32, tag="tmpj")
            nc.vector.tensor_mul(tmp, ou, v_view)
            dotu = small.tile([P, 4, 1], f32, tag="dotu")
            nc.vector.reduce_sum(dotu, tmp, axis=mybir.AxisListType.X)
            cu = small.tile([P, 4, 1], f32, tag="cu")
            nc.vector.tensor_mul(cu, dotu, rvns[:, cs])
            nc.vector.tensor_mul(tmp, v_view, cu.to_broadcast([P, 4, HEAD_DIM]))
            nc.vector.tensor_sub(ou, ou, tmp)
            nc.vector.tensor_mul(ou, ou, rden.to_broadcast([P, 4, HEAD_DIM]))

        def ftr_j(pair, ot, j):
            """Transpose corrected O chunks 4j..4j+3 back into OT rows —
            returned as filler thunks."""
            thunks = []
            for c in range(4 * j, 4 * j + 4):
                for h in range(2):
                    def th(c=c, h=h):
                        o_h = o_tiles[pair * 2 + h]
                        lo = h * HEAD_DIM
                        pt2 = psum_t.tile(
                            [P, 4 * (HEAD_DIM + 1)], f32, tag="pt", name="pt2"
                        )
                        nc.tensor.transpose(
                            pt2[0:HEAD_DIM, 0:P], o_h[:, c, 0:HEAD_DIM], ident
                        )
                        nc.vector.tensor_copy(
                            out=ot[lo : lo + HEAD_DIM, c * P : (c + 1) * P],
                            in_=pt2[0:HEAD_DIM, 0:P],
                        )
                    thunks.append(th)
            return thunks

        def attention_pair(pair, qt, kt, ot):
            """Both heads together: the two K=64 score matmuls use disjoint
            PE row groups (base partitions 0/64) and run concurrently.
            Fillers (remaining projections, pending output transposes, and —
            for pair 1 — the incremental output projection) are drip-fed to
            absorb exp-wait stalls and keep the PE clock warm. drain()
            enforces that any filler writing a region this loop reads has
            been emitted first."""
            o_hA = o_pool.tile([P, SCH, HEAD_DIM + 1], f32, tag="oh")
            o_hB = o_pool.tile([P, SCH, HEAD_DIM + 1], f32, tag="oh")
            o_tiles[pair * 2] = o_hA
            o_tiles[pair * 2 + 1] = o_hB
            rv_state = {}

            def ensure_rv():
                if not rv_state:
                    for c in range(SCH):
                        drain(("V", c))
                    rv_state[0] = head_setup(pair * 2)
                    rv_state[1] = head_setup(pair * 2 + 1)

            for j in range(NQB):
                drain(("Q", pair, j))
                po_A = psum_o.tile([HEAD_DIM + 1, QB], f32, tag="po")
                po_B = psum_o.tile([HEAD_DIM + 1, QB], f32, tag="po")

                def scores(c):
                    s2 = psum_s.tile([P, 2 * QB], f32, tag="s2")
                    for h, lo in ((0, 0), (1, HEAD_DIM)):
                        nc.tensor.matmul(
                            s2[:, h * QB : (h + 1) * QB],
                            lhsT=kt[lo : lo + HEAD_DIM, c * P : (c + 1) * P],
                            rhs=qt[lo : lo + HEAD_DIM, j * QB : (j + 1) * QB],
                            start=True,
                            stop=True,
                        )
                    e2 = e_pool.tile([P, 2 * QB], mm_dt, tag="e2")
                    nc.scalar.activation(e2, s2, AF.Exp, scale=float(SCALE))
                    return e2

                def pv(c, e2):
                    for po, h in ((po_A, 0), (po_B, 1)):
                        nc.tensor.matmul(
                            po,
                            lhsT=v_sb[:, c, pair * 2 + h, :],
                            rhs=e2[:, h * QB : (h + 1) * QB],
                            start=(c == 0),
                            stop=(c == SCH - 1),
                            skip_group_check=True,
                        )

                prev_e2 = None
                for c in range(SCH):
                    drain(("K", pair, c // 4))
                    e2 = scores(c)
                    if prev_e2 is not None:
                        drain(("V", c - 1))
                        pv(c - 1, prev_e2)
                    L = len(filler_q)
                    if pair == 0:
                        if j > 0:
                            pop_filler(2 if L > 48 else 1)
                    else:
                        # pair 1: ftr + output-projection fillers, no
                        # concurrent input DMA — ~3/chunk ran at full duty.
                        # Drain aggressively: every thunk left at the end
                        # runs in a half-duty-clocked tail instead.
                        pop_filler(3)
                    prev_e2 = e2
                drain(("V", SCH - 1))
                pv(SCH - 1, prev_e2)

                for po, o_h in ((po_A, o_hA), (po_B, o_hB)):
                    otu = otu_pool.tile([HEAD_DIM + 1, QB], f32, tag="otu")
                    nc.vector.tensor_copy(otu, po)
                    pt = psum_t.tile([P, 4 * (HEAD_DIM + 1)], f32, tag="pt")
                    for i in range(4):
                        nc.tensor.transpose(
                            pt[:, i * 65 : (i + 1) * 65],
                            otu[:, i * P : (i + 1) * P],
                            ident[0:65, 0:65],
                        )
                    nc.vector.tensor_copy(
                        out=o_h[:, j * 4 : (j + 1) * 4, :],
                        in_=pt.rearrange("p (i e) -> p i e", i=4),
                    )
                ensure_rv()
                correct_j(pair * 2, j, rv_state[0])
                correct_j(pair * 2 + 1, j, rv_state[1])
                enqueue_unit(("F", pair, j), ftr_j(pair, ot, j))
                if pair == 1:
                    # OT columns 4j..4j+3 of both pairs are now queued for
                    # transpose; project + DMA them behind those fillers.
                    for sc in range(4 * j, 4 * j + 4):
                        enqueue_unit(("O", sc), outproj_unit(sc))

        # ---------------- preamble: just enough to start attention ----------------
        # inline K0 block 0, V chunks 0-3, Q0 block 0; the rest of V/K0
        # drips through the filler queue, drain-gated at its first reader
        # inside j0's c-loop (j0 pops nothing extra, so the in-order PE
        # queue never runs ahead of the next j-block's scores).
        for th in project_block(wk_sb, kt0, 0, 0, pool=psum_s):
            th()
        for c in range(4):
            v_chunk_inline(c)
        for th in project_block(wq_sb, qt0, 0, 0, pool=psum_s):
            th()

        enqueue_unit(("K", 0, 1), project_block(wk_sb, kt0, 0, 1))
        for c in range(4, 8):
            enqueue_unit(("V", c), v_chunk_thunks(c))
        for blk in range(2, NQB):
            enqueue_unit(("K", 0, blk), project_block(wk_sb, kt0, 0, blk))
            for c in range(4 * blk, 4 * blk + 4):
                enqueue_unit(("V", c), v_chunk_thunks(c))
        for blk in range(1, NQB):
            enqueue_unit(("Q", 0, blk), project_block(wq_sb, qt0, 0, blk))
        for blk in range(NQB):
            enqueue_unit(("K", 1, blk), project_block(wk_sb, kt1, 1, blk))
        for blk in range(NQB):
            enqueue_unit(("Q", 1, blk), project_block(wq_sb, qt1, 1, blk))

        attention_pair(0, qt0, kt0, ot0)
        attention_pair(1, qt1, kt1, ot1)

        # tail: flush pending transposes + the last output-projection chunks
        while filler_q:
            pop_one()

    if fixup:
        _fixup_sync_waits(nc)
    return nc


def _get_program():
    global _PROGRAM
    if _PROGRAM is None:
        _PROGRAM = _build_program()
    return _PROGRAM


def _make_in_maps(x, Wq, Wk, Wv, Wo, xsa_scale):
    in_np = np.float16 if USE_FP16 else np.float32
    x = np.ascontiguousarray(np.asarray(x, dtype=np.float32))
    Wq = np.ascontiguousarray(np.asarray(Wq, dtype=np.float32))
    Wk = np.ascontiguousarray(np.asarray(Wk, dtype=np.float32))
    Wv = np.ascontiguousarray(np.asarray(Wv, dtype=np.float32))
    Wo = np.ascontiguousarray(np.asarray(Wo, dtype=np.float32))
    gamma = np.full((P, 1), np.float32(np.asarray(xsa_scale).reshape(-1)[0]))

    in_maps = []
    for core in range(N_CORES):
        b = core // 4
        g = core % 4
        cs = slice(g * CD, (g + 1) * CD)
        in_maps.append(
            {
                "xt": np.ascontiguousarray(x[b].T.astype(in_np)),
                "wqt": np.ascontiguousarray(Wq[cs, :].T.astype(in_np)),
                "wkt": np.ascontiguousarray(Wk[cs, :].T.astype(in_np)),
                "wvt": np.ascontiguousarray(Wv[cs, :].T.astype(in_np)),
                "wot": np.ascontiguousarray(Wo[:, cs].T.astype(in_np)),
                "gamma": gamma,
            }
        )
    return in_maps


def _ensure_ntff_hook():
    """The agent image's antenv lacks axon_hooks; reconstruct it so
    run_bass_kernel_spmd(trace=True) can capture NTFF profiles."""
    import sys as _sys
    import types

    if "antenv.axon_hooks" in _sys.modules:
        return
    mod = types.ModuleType("antenv.axon_hooks")
    state = {"hook": None}
    mod.set_axon_ntff_profile_hook = lambda h: state.__setitem__("hook", h)
    mod.get_axon_ntff_profile_hook = lambda: state["hook"]
    _sys.modules["antenv.axon_hooks"] = mod
    try:
        import antenv

        antenv.axon_hooks = mod
    except ImportError:
        pass
    try:
        from trn_agent_boot.trn_boot import _ntff_profile_via_ctypes

        mod.set_axon_ntff_profile_hook(
            _ntff_profile_via_ctypes("/opt/axon/libaxon_pjrt.so")
        )
    except Exception as e:  # degrade to no-trace
        print(f"ntff hook setup failed ({e}); tracing disabled", file=sys.stderr)


def _run(inputs, trace=False, **kwargs):
    from concourse.bass_utils import run_bass_kernel_spmd

    if trace:
        _ensure_ntff_hook()

    nc = _get_program()
    in_maps = _make_in_maps(
        inputs["x"], inputs["Wq"], inputs["Wk"], inputs["Wv"], inputs["Wo"],
        inputs["xsa_scale"],
    )
    res = run_bass_kernel_spmd(
        nc, in_maps, core_ids=list(range(N_CORES)), trace=trace, **kwargs
    )
    parts = [np.asarray(res.results[c]["out_p"]) for c in range(N_CORES)]
    out = np.stack(
        [
            parts[0] + parts[1] + parts[2] + parts[3],
            parts[4] + parts[5] + parts[6] + parts[7],
        ]
    ).astype(np.float32)
    return out, res


def kernel(**inputs) -> np.ndarray:
    out, _ = _run(inputs, trace=False)
    return out


# revision 22
# speedup vs baseline: 1.3514x; 1.0253x over previous
"""ExclusiveSelfAttention Trainium2 kernel (8-core SPMD, Megatron-style head TP).

Sharding: core c -> batch b = c // 4, head group g = c % 4 (heads 4g..4g+3).
Each core computes its 4 heads' attention plus the partial Wo projection for
its batch; the host sums the 4 partials per batch (the row-parallel
all-reduce, done at unshard time).

Device layout choices:
  - x is shipped pre-transposed (XT: [D, S]) so the contraction dim d sits on
    SBUF partitions for the QKV projections.
  - Q,K are produced directly transposed ([2*64 head-pair dims, S]) so the
    scores matmul needs no further transposes.
  - Scores are computed transposed (S^T[k, q]) so softmax-exp feeds the PV
    matmul directly; V carries an appended ones-column so PV row 64
    accumulates the softmax denominator for free.
  - exp skips max-subtraction (logits ~ N(0,1); exact same math in fp32).
  - The [65, 512] PV output is PE-transposed back to natural [q, 65] layout
    for the exclusive-projection correction, which then runs as a handful of
    batched DVE ops per head.

Scheduling (v2): the kernel streams from the first microsecond instead of
serializing load -> project -> attend -> project-out:
  - x arrives in [128, 512] column pieces; V-chunks / K / Q projection blocks
    are emitted as soon as their pieces land.
  - attention pair 0 starts after just {K0 block 0, V chunks 0-3, Q0 block 0};
    the remaining V/K0/Q0 work plus all of pair-1's projections drain through
    the filler queue inside the attention c-loop. Explicit drain-to-boundary
    bookkeeping guarantees every filler that writes a region is emitted
    before the instruction that reads it.
  - the output projection runs incrementally: as soon as pair-1's j-block is
    corrected and transposed, its four OT column chunks are projected through
    Wo and DMA'd out as fillers, so the kernel has no cold tail (the HAM
    down-clocks the core to half speed when PE activity drops; the v1 tail
    ran ~46us at half clock).

Walrus on TRN2 rejects instructions carrying too many semaphore waits
(Matmult: >1, others: >4 — "Too many sync wait commands"). _fixup_sync_waits
post-processes the scheduled BIR: excess waits are spilled onto NoOp
instructions inserted just before the offender on the same engine, which is
semantically identical (the waits still execute, in order, before it).
"""

import sys

if "/opt/trn_rl_repo" not in sys.path:
    sys.path.insert(0, "/opt/trn_rl_repo")

import numpy as np

NUM_HEADS = 16
HEAD_DIM = 64
D = NUM_HEADS * HEAD_DIM  # 1024
B = 2
S = 2048
EPS = 1e-8
SCALE = 1.0 / np.sqrt(HEAD_DIM)  # 0.125

N_CORES = 8
HPC = 4  # heads per core
CD = HPC * HEAD_DIM  # per-core slice of the model dim: 256
P = 128
DCH = D // P  # 8 d-chunks
SCH = S // P  # 16 sequence chunks
QB = 512  # query block
NQB = S // QB  # 4

_PROGRAM = None  # cached Bass program

# fp16 streams through the PE at 1 cycle/column (like bf16) with 10 mantissa
# bits — simulated end-to-end precision 7.4e-4 scale-relative vs the f64
# reference (fp32r baseline: 3.4e-4; bf16: 6.5e-3). fp16 operands need no
# f32r-style "produced by a compute engine" rounding, so inputs DMA straight
# into their tiles with no bounce buffers or casts, and the 2-byte streams
# halve DMA bytes and PE input energy (headroom for the HAM power governor).
USE_FP16 = True


def _fixup_sync_waits(nc):
    """Spill semaphore waits beyond walrus's per-instruction limits onto
    NoOps inserted before the offending instruction (same engine)."""
    from concourse import mybir

    n_spill = 0
    for fn in nc.m.functions:
        for bb in fn.blocks:
            il = bb.instructions
            i = 0
            while i < len(il):
                ins = il[i]
                si = ins.sync_info
                if si is None or not si.on_wait:
                    i += 1
                    continue
                waits = list(si.on_wait)
                sem_idx = [
                    k for k, w in enumerate(waits)
                    if getattr(w, "sync_type", "semaphore") == "semaphore"
                ]
                if isinstance(ins, mybir.InstAllEngineBarrier):
                    i += 1
                    continue
                limit = 1  # this walrus allows one sem wait per instruction
                if len(waits) <= limit:
                    i += 1
                    continue
                n_excess = len(waits) - limit
                spill = [waits[k] for k in sem_idx[:n_excess]]
                if len(spill) < n_excess:
                    i += 1
                    continue  # non-semaphore waits; leave untouched
                keep = [w for w in waits if not any(w is s for s in spill)]
                pos = i
                for w in spill:  # one wait per NoOp — safest across opcodes
                    nop = mybir.InstNoOp(
                        name=f"I-wspill-{n_spill}",
                        text_hint="wait_spill",
                        bass_nofuse=True,
                    )
                    n_spill += 1
                    nop.engine = ins.engine
                    nop.sync_info = mybir.SyncInfo(on_wait=[w], on_update=[])
                    il.insert(pos, nop)
                    pos += 1
                    i += 1
                si.on_wait = keep
                i += 1
    return n_spill


def _build_program(fixup=True):
    import concourse.bass as bass
    import concourse.tile as tile
    from concourse import mybir
    from concourse.tile import add_dep_helper
    from contextlib import ExitStack
    from collections import deque

    f32 = mybir.dt.float32
    f16 = mybir.dt.float16
    AF = mybir.ActivationFunctionType

    nc = bass.Bass("TRN2", target_bir_lowering=False, debug=False)

    in_dt = mybir.dt.float16 if USE_FP16 else mybir.dt.float32
    xt_d = nc.dram_tensor("xt", [D, S], in_dt, kind="ExternalInput").ap()
    wqt_d = nc.dram_tensor("wqt", [D, CD], in_dt, kind="ExternalInput").ap()
    wkt_d = nc.dram_tensor("wkt", [D, CD], in_dt, kind="ExternalInput").ap()
    wvt_d = nc.dram_tensor("wvt", [D, CD], in_dt, kind="ExternalInput").ap()
    wot_d = nc.dram_tensor("wot", [CD, D], in_dt, kind="ExternalInput").ap()
    gamma_d = nc.dram_tensor("gamma", [P, 1], f32, kind="ExternalInput").ap()
    out_d = nc.dram_tensor("out_p", [S, D], f32, kind="ExternalOutput").ap()

    mm_dt = f16 if USE_FP16 else f32

    with tile.TileContext(nc) as tc, ExitStack() as ctx:
        pers = ctx.enter_context(tc.tile_pool(name="pers", bufs=1))
        qk_pool = ctx.enter_context(tc.tile_pool(name="qk", bufs=1))
        e_pool = ctx.enter_context(tc.tile_pool(name="e", bufs=3))
        otu_pool = ctx.enter_context(tc.tile_pool(name="otu", bufs=2))
        o_pool = ctx.enter_context(tc.tile_pool(name="o", bufs=4))
        ot_pool = ctx.enter_context(tc.tile_pool(name="ot", bufs=2))
        small = ctx.enter_context(tc.tile_pool(name="small", bufs=2))
        tmp_pool = ctx.enter_context(tc.tile_pool(name="tmp", bufs=1))
        stage_pool = ctx.enter_context(tc.tile_pool(name="stage", bufs=3))
        # PSUM budget (8 banks): s2 [128,1024] x2 = 4, po x2 = 2,
        # prj (projection / output-projection fillers) = 1, pt (transposes) = 1.
        psum_s = ctx.enter_context(tc.tile_pool(name="ps_s", bufs=2, space="PSUM"))
        psum_o = ctx.enter_context(tc.tile_pool(name="ps_o", bufs=2, space="PSUM"))
        psum_prj = ctx.enter_context(tc.tile_pool(name="ps_prj", bufs=1, space="PSUM"))
        psum_t = ctx.enter_context(tc.tile_pool(name="ps_t", bufs=1, space="PSUM"))

        ident = pers.tile([P, P], f32, tag="ident")
        nc.gpsimd.memset(ident, 0.0)
        nc.gpsimd.affine_select(
            out=ident,
            in_=ident,
            compare_op=mybir.AluOpType.not_equal,
            fill=1.0,
            base=0,
            pattern=[[-1, P]],
            channel_multiplier=1,
        )
        gamma = pers.tile([P, 1], f32, tag="gamma")
        nc.sync.dma_start(gamma, gamma_d)

        # ---------------- input streaming ----------------
        # fp16 operands have no walrus "rounded by a compute engine" rule
        # (that is fp32r-only), so inputs DMA straight into their final
        # tiles: no bounce buffers, no casts, nothing to ping-pong against.
        dma_state = {"n": 0}

        def dma_eng():
            eng = nc.sync if dma_state["n"] % 2 == 0 else nc.scalar
            dma_state["n"] += 1
            return eng

        def load_direct(dst_slice, src_ap):
            return dma_eng().dma_start(dst_slice, src_ap)

        wq_sb = pers.tile([P, DCH, CD], mm_dt, tag="wq")
        wk_sb = pers.tile([P, DCH, CD], mm_dt, tag="wk")
        wv_sb = pers.tile([P, DCH, CD], mm_dt, tag="wv")
        wo_sb = pers.tile([P, CD // P, D], mm_dt, tag="wo")
        xt_sb = [
            pers.tile([P, S], mm_dt, tag=f"xt{d}", name=f"xt_sb{d}")
            for d in range(DCH)
        ]
        x_dma = {}

        def load_x_half(d, half):
            x_dma[(d, half)] = load_direct(
                xt_sb[d][:, half * (S // 2) : (half + 1) * (S // 2)],
                xt_d[d * P : (d + 1) * P, half * (S // 2) : (half + 1) * (S // 2)],
            )

        # arrival order = need order: wv/wk (V + K0 first), x first halves,
        # wq, x second halves, wo (only needed by the output projection).
        load_direct(wv_sb, wvt_d.rearrange("(o p) e -> p o e", p=P))
        load_direct(wk_sb, wkt_d.rearrange("(o p) e -> p o e", p=P))
        for d in range(DCH):
            load_x_half(d, 0)
        load_direct(wq_sb, wqt_d.rearrange("(o p) e -> p o e", p=P))
        for d in range(DCH):
            load_x_half(d, 1)
        load_direct(wo_sb, wot_d.rearrange("(o p) f -> p o f", p=P))

        # preload the ACT exp table while DMAs run
        exp_warm = stage_pool.tile([P, P], f32, tag="expwarm")
        nc.scalar.activation(exp_warm, ident, AF.Exp, scale=0.01)

        # HAM warmup: a short dense matmul burst gated on an early x piece,
        # so the PE array is at speed when the first projections start
        for w in range(8):
            wm = psum_s.tile([P, P], f32, tag="s2")
            mi = nc.tensor.matmul(wm, lhsT=ident, rhs=ident, start=True, stop=True)
            if w == 0:
                add_dep_helper(mi.ins, x_dma[(2, 0)].ins, reason="warmup gate")

        # V in natural layout [k-chunk, head, 65], col 64 = ones (denominator trick)
        v_sb = pers.tile([P, SCH, HPC, HEAD_DIM + 1], mm_dt, tag="v")
        ones_col = pers.tile([P, 1], f32, tag="ones")
        nc.vector.memset(ones_col, 1.0)
        nc.vector.tensor_copy(
            out=v_sb[:, :, :, HEAD_DIM : HEAD_DIM + 1],
            in_=ones_col[:, None, :, None].to_broadcast([P, SCH, HPC, 1]),
        )

        # ---------------- filler queue with drain boundaries ----------------
        filler_q = deque()
        outstanding = {}

        def enqueue_unit(uid, thunks):
            if uid is not None:
                outstanding[uid] = outstanding.get(uid, 0) + len(thunks)
            for th in thunks:
                filler_q.append((uid, th))

        def pop_one():
            if not filler_q:
                return False
            uid, th = filler_q.popleft()
            th()
            if uid is not None:
                outstanding[uid] -= 1
            return True

        def pop_filler(n=1):
            for _ in range(n):
                if not pop_one():
                    return

        def drain(uid):
            while outstanding.get(uid, 0) > 0:
                if not pop_one():
                    raise RuntimeError(f"drain {uid}: queue underflow")

        def v_chunk_inline(c):
            """One V chunk: 8 accumulating matmuls + natural-layout copy.
            Runs in the double-buffered s2 ring (preamble only)."""
            pv = psum_s.tile([P, CD], f32, tag="s2", name="pv")
            for d in range(DCH):
                nc.tensor.matmul(
                    pv,
                    lhsT=xt_sb[d][:, c * P : (c + 1) * P],
                    rhs=wv_sb[:, d, :],
                    start=(d == 0),
                    stop=(d == DCH - 1),
                )
            nc.vector.tensor_copy(
                out=v_sb[:, c, :, 0:HEAD_DIM],
                in_=pv.rearrange("p (h e) -> p h e", h=HPC),
            )

        def v_chunk_thunks(c):
            """Filler-queue variant of a V chunk, through the shared prj
            bank (popped inside attention pair 0's c-loop)."""
            state = {}

            def mk(d):
                def run():
                    if d == 0:
                        state["pv"] = psum_prj.tile([P, CD], f32, tag="prj", name="pv")
                    nc.tensor.matmul(
                        state["pv"],
                        lhsT=xt_sb[d][:, c * P : (c + 1) * P],
                        rhs=wv_sb[:, d, :],
                        start=(d == 0),
                        stop=(d == DCH - 1),
                        skip_group_check=True,
                    )
                return run

            thunks = [mk(d) for d in range(DCH)]

            def cp():
                nc.vector.tensor_copy(
                    out=v_sb[:, c, :, 0:HEAD_DIM],
                    in_=state["pv"].rearrange("p (h e) -> p h e", h=HPC),
                )

            thunks.append(cp)
            return thunks

        def project_block(wsb, dst, pair, blk, pool=None):
            """Thunks for one [128,512] projection block: 8 accumulating
            matmuls into a PSUM bank + the copy-out. Fillers use the
            dedicated prj bank; the inline preamble passes psum_s."""
            state = {}
            pool = pool or psum_prj
            tag = "s2" if pool is psum_s else "prj"

            def mk_mm(d):
                def run():
                    if d == 0:
                        state["pq"] = pool.tile([P, QB], f32, tag=tag, name="pq_prj")
                    nc.tensor.matmul(
                        state["pq"],
                        lhsT=wsb[:, d, pair * P : (pair + 1) * P],
                        rhs=xt_sb[d][:, blk * QB : (blk + 1) * QB],
                        start=(d == 0),
                        stop=(d == DCH - 1),
                        skip_group_check=True,
                    )
                return run

            thunks = [mk_mm(d) for d in range(DCH)]

            def cp():
                nc.vector.tensor_copy(
                    out=dst[:, blk * QB : (blk + 1) * QB], in_=state["pq"]
                )

            thunks.append(cp)
            return thunks

        ot0 = ot_pool.tile([P, S], mm_dt, tag="ot")
        ot1 = ot_pool.tile([P, S], mm_dt, tag="ot")
        ot_tiles = [ot0, ot1]
        qt0 = qk_pool.tile([P, S], mm_dt, tag="qt")
        kt0 = qk_pool.tile([P, S], mm_dt, tag="kt")
        qt1 = qk_pool.tile([P, S], mm_dt, tag="qt1")
        kt1 = qk_pool.tile([P, S], mm_dt, tag="kt1")
        qk_t = {(0, "q"): qt0, (0, "k"): kt0, (1, "q"): qt1, (1, "k"): kt1}

        def outproj_unit(sc):
            """Partial output projection + DMA for OT column chunk sc:
            out_p[s, f] = sum_c OT[c, s] * WoT[c, f]."""
            ths = []
            for fb in range(2):
                st = {}

                def m0(sc=sc, fb=fb, st=st):
                    st["pp"] = psum_prj.tile([P, QB], f32, tag="prj", name="pp")
                    nc.tensor.matmul(
                        st["pp"],
                        lhsT=ot0[:, sc * P : (sc + 1) * P],
                        rhs=wo_sb[:, 0, fb * QB : (fb + 1) * QB],
                        start=True,
                        stop=False,
                        skip_group_check=True,
                    )

                def m1(sc=sc, fb=fb, st=st):
                    nc.tensor.matmul(
                        st["pp"],
                        lhsT=ot1[:, sc * P : (sc + 1) * P],
                        rhs=wo_sb[:, 1, fb * QB : (fb + 1) * QB],
                        start=False,
                        stop=True,
                        skip_group_check=True,
                    )

                def cp(sc=sc, fb=fb, st=st):
                    stg = stage_pool.tile([P, QB], f32, tag="stage")
                    nc.vector.tensor_copy(stg, st["pp"])
                    nc.sync.dma_start(
                        out_d[sc * P : (sc + 1) * P, fb * QB : (fb + 1) * QB], stg
                    )

                ths += [m0, m1, cp]
            return ths

        o_tiles = {}  # hg -> o_h tile

        def head_setup(hg):
            """Per-head vns/rvns (depends only on V): gamma/vns folded."""
            v_view = v_sb[:, :, hg, 0:HEAD_DIM]
            tmp = tmp_pool.tile([P, SCH, HEAD_DIM], f32, tag="tmp")
            nc.gpsimd.tensor_mul(tmp, v_view, v_view)
            vns = small.tile([P, SCH, 1], f32, tag=f"vns{hg % 2}", name="vns")
            nc.vector.reduce_sum(vns, tmp, axis=mybir.AxisListType.X)
            nc.vector.tensor_scalar_add(vns, vns, float(EPS))
            rvns = small.tile([P, SCH, 1], f32, tag=f"rvns{hg % 2}", name="rvns")
            nc.vector.reciprocal(rvns, vns)
            nc.vector.tensor_scalar_mul(rvns, rvns, gamma)
            return rvns

        def correct_j(hg, j, rvns):
            """Correction for q-chunks 4j..4j+3 of head hg (runs on DVE
            while later q-blocks are still in the matmul pipeline)."""
            o_h = o_tiles[hg]
            cs = slice(4 * j, 4 * j + 4)
            v_view = v_sb[:, cs, hg, 0:HEAD_DIM]
            ou = o_h[:, cs, 0:HEAD_DIM]
            den = o_h[:, cs, HEAD_DIM : HEAD_DIM + 1]
            rden = small.tile([P, 4, 1], f32, tag="rden")
            nc.vector.reciprocal(rden, den)
            tmp = tmp_pool.tile([P, 4, HEAD_DIM], f32, tag="tmpj")
            nc.vector.tensor_mul(tmp, ou, v_view)
            dotu = small.tile([P, 4, 1], f32, tag="dotu")
            nc.vector.reduce_sum(dotu, tmp, axis=mybir.AxisListType.X)
            cu = small.tile([P, 4, 1], f32, tag="cu")
            nc.vector.tensor_mul(cu, dotu, rvns[:, cs])
            nc.vector.tensor_mul(tmp, v_view, cu.to_broadcast([P, 4, HEAD_DIM]))
            nc.vector.tensor_sub(ou, ou, tmp)
            nc.vector.tensor_mul(ou, ou, rden.to_broadcast([P, 4, HEAD_DIM]))

        def ftr_j(pair, ot, j):
            """Transpose corrected O chunks 4j..4j+3 back into OT rows —
            returned as filler thunks."""
            thunks = []
            for c in range(4 * j, 4 * j + 4):
                for h in range(2):
                    def th(c=c, h=h):
                        o_h = o_tiles[pair * 2 + h]
                        lo = h * HEAD_DIM
                        pt2 = psum_t.tile(
                            [P, 4 * (HEAD_DIM + 1)], f32, tag="pt", name="pt2"
                        )
                        nc.tensor.transpose(
                            pt2[0:HEAD_DIM, 0:P], o_h[:, c, 0:HEAD_DIM], ident
                        )
                        nc.vector.tensor_copy(
                            out=ot[lo : lo + HEAD_DIM, c * P : (c + 1) * P],
                            in_=pt2[0:HEAD_DIM, 0:P],
                        )
                    thunks.append(th)
            return thunks

        def attention_pair(pair, qt, kt, ot):
            """Both heads together: the two K=64 score matmuls use disjoint
            PE row groups (base partitions 0/64) and run concurrently.
            Fillers (remaining projections, pending output transposes, and —
            for pair 1 — the incremental output projection) are drip-fed to
            absorb exp-wait stalls and keep the PE clock warm. drain()
            enforces that any filler writing a region this loop reads has
            been emitted first."""
            o_hA = o_pool.tile([P, SCH, HEAD_DIM + 1], f32, tag="oh")
            o_hB = o_pool.tile([P, SCH, HEAD_DIM + 1], f32, tag="oh")
            o_tiles[pair * 2] = o_hA
            o_tiles[pair * 2 + 1] = o_hB
            rv_state = {}

            def ensure_rv():
                if not rv_state:
                    for c in range(SCH):
                        drain(("V", c))
                    rv_state[0] = head_setup(pair * 2)
                    rv_state[1] = head_setup(pair * 2 + 1)

            for j in range(NQB):
                drain(("Q", pair, j))
                po_A = psum_o.tile([HEAD_DIM + 1, QB], f32, tag="po")
                po_B = psum_o.tile([HEAD_DIM + 1, QB], f32, tag="po")

                def scores(c):
                    s2 = psum_s.tile([P, 2 * QB], f32, tag="s2")
                    for h, lo in ((0, 0), (1, HEAD_DIM)):
                        nc.tensor.matmul(
                            s2[:, h * QB : (h + 1) * QB],
                            lhsT=kt[lo : lo + HEAD_DIM, c * P : (c + 1) * P],
                            rhs=qt[lo : lo + HEAD_DIM, j * QB : (j + 1) * QB],
                            start=True,
                            stop=True,
                        )
                    e2 = e_pool.tile([P, 2 * QB], mm_dt, tag="e2")
                    nc.scalar.activation(e2, s2, AF.Exp, scale=float(SCALE))
                    return e2

                def pv(c, e2):
                    for po, h in ((po_A, 0), (po_B, 1)):
                        nc.tensor.matmul(
                            po,
                            lhsT=v_sb[:, c, pair * 2 + h, :],
                            rhs=e2[:, h * QB : (h + 1) * QB],
                            start=(c == 0),
                            stop=(c == SCH - 1),
                            skip_group_check=True,
                        )

                prev_e2 = None
                for c in range(SCH):
                    drain(("K", pair, c // 4))
                    drain(("V", min(c + 1, SCH - 1)))
                    if c == SCH - 2 and j + 1 < NQB:
                        drain(("Q", pair, j + 1))
                    e2 = scores(c)
                    if prev_e2 is not None:
                        pv(c - 1, prev_e2)
                    L = len(filler_q)
                    if pair == 0:
                        if j > 0:
                            pop_filler(2 if L > 48 else 1)
                    else:
                        # pair 1: ftr + output-projection fillers, no
                        # concurrent input DMA — ~3/chunk ran at full duty.
                        # Drain aggressively: every thunk left at the end
                        # runs in a half-duty-clocked tail instead.
                        pop_filler(3)
                    prev_e2 = e2
                pv(SCH - 1, prev_e2)

                for po, o_h in ((po_A, o_hA), (po_B, o_hB)):
                    otu = otu_pool.tile([HEAD_DIM + 1, QB], f32, tag="otu")
                    nc.vector.tensor_copy(otu, po)
                    pt = psum_t.tile([P, 4 * (HEAD_DIM + 1)], f32, tag="pt")
                    for i in range(4):
                        nc.tensor.transpose(
                            pt[:, i * 65 : (i + 1) * 65],
                            otu[:, i * P : (i + 1) * P],
                            ident[0:65, 0:65],
                        )
                    nc.vector.tensor_copy(
                        out=o_h[:, j * 4 : (j + 1) * 4, :],
                        in_=pt.rearrange("p (i e) -> p i e", i=4),
                    )
                ensure_rv()
                correct_j(pair * 2, j, rv_state[0])
                correct_j(pair * 2 + 1, j, rv_state[1])
                enqueue_unit(("F", pair, j), ftr_j(pair, ot, j))
                if pair == 1:
                    # OT columns 4j..4j+3 of both pairs are now queued for
                    # transpose; project + DMA them behind those fillers.
                    for sc in range(4 * j, 4 * j + 4):
                        enqueue_unit(("O", sc), outproj_unit(sc))

        # ---------------- preamble: just enough to start attention ----------------
        # inline K0 block 0, V chunks 0-3, Q0 block 0; the rest of V/K0
        # drips through the filler queue, drain-gated at its first reader
        # inside j0's c-loop (j0 pops nothing extra, so the in-order PE
        # queue never runs ahead of the next j-block's scores).
        for th in project_block(wk_sb, kt0, 0, 0, pool=psum_s):
            th()
        for c in range(4):
            v_chunk_inline(c)
        for th in project_block(wq_sb, qt0, 0, 0, pool=psum_s):
            th()

        enqueue_unit(("K", 0, 1), project_block(wk_sb, kt0, 0, 1))
        for c in range(4, 8):
            enqueue_unit(("V", c), v_chunk_thunks(c))
        for blk in range(2, NQB):
            enqueue_unit(("K", 0, blk), project_block(wk_sb, kt0, 0, blk))
            for c in range(4 * blk, 4 * blk + 4):
                enqueue_unit(("V", c), v_chunk_thunks(c))
        for blk in range(1, NQB):
            enqueue_unit(("Q", 0, blk), project_block(wq_sb, qt0, 0, blk))
        for blk in range(NQB):
            enqueue_unit(("K", 1, blk), project_block(wk_sb, kt1, 1, blk))
        for blk in range(NQB):
            enqueue_unit(("Q", 1, blk), project_block(wq_sb, qt1, 1, blk))

        attention_pair(0, qt0, kt0, ot0)
        attention_pair(1, qt1, kt1, ot1)

        # tail: flush pending transposes + the last output-projection chunks
        while filler_q:
            pop_one()

    if fixup:
        _fixup_sync_waits(nc)
    return nc


def _get_program():
    global _PROGRAM
    if _PROGRAM is None:
        _PROGRAM = _build_program()
    return _PROGRAM


def _make_in_maps(x, Wq, Wk, Wv, Wo, xsa_scale):
    in_np = np.float16 if USE_FP16 else np.float32
    x = np.ascontiguousarray(np.asarray(x, dtype=np.float32))
    Wq = np.ascontiguousarray(np.asarray(Wq, dtype=np.float32))
    Wk = np.ascontiguousarray(np.asarray(Wk, dtype=np.float32))
    Wv = np.ascontiguousarray(np.asarray(Wv, dtype=np.float32))
    Wo = np.ascontiguousarray(np.asarray(Wo, dtype=np.float32))
    gamma = np.full((P, 1), np.float32(np.asarray(xsa_scale).reshape(-1)[0]))

    in_maps = []
    for core in range(N_CORES):
        b = core // 4
        g = core % 4
        cs = slice(g * CD, (g + 1) * CD)
        in_maps.append(
            {
                "xt": np.ascontiguousarray(x[b].T.astype(in_np)),
                "wqt": np.ascontiguousarray(Wq[cs, :].T.astype(in_np)),
                "wkt": np.ascontiguousarray(Wk[cs, :].T.astype(in_np)),
                "wvt": np.ascontiguousarray(Wv[cs, :].T.astype(in_np)),
                "wot": np.ascontiguousarray(Wo[:, cs].T.astype(in_np)),
                "gamma": gamma,
            }
        )
    return in_maps


def _ensure_ntff_hook():
    """The agent image's antenv lacks axon_hooks; reconstruct it so
    run_bass_kernel_spmd(trace=True) can capture NTFF profiles."""
    import sys as _sys
    import types

    if "antenv.axon_hooks" in _sys.modules:
        return
    mod = types.ModuleType("antenv.axon_hooks")
    state = {"hook": None}
    mod.set_axon_ntff_profile_hook = lambda h: state.__setitem__("hook", h)
    mod.get_axon_ntff_profile_hook = lambda: state["hook"]
    _sys.modules["antenv.axon_hooks"] = mod
    try:
        import antenv

        antenv.axon_hooks = mod
    except ImportError:
        pass
    try:
        from trn_agent_boot.trn_boot import _ntff_profile_via_ctypes

        mod.set_axon_ntff_profile_hook(
            _ntff_profile_via_ctypes("/opt/axon/libaxon_pjrt.so")
        )
    except Exception as e:  # degrade to no-trace
        print(f"ntff hook setup failed ({e}); tracing disabled", file=sys.stderr)


def _run(inputs, trace=False, **kwargs):
    from concourse.bass_utils import run_bass_kernel_spmd

    if trace:
        _ensure_ntff_hook()

    nc = _get_program()
    in_maps = _make_in_maps(
        inputs["x"], inputs["Wq"], inputs["Wk"], inputs["Wv"], inputs["Wo"],
        inputs["xsa_scale"],
    )
    res = run_bass_kernel_spmd(
        nc, in_maps, core_ids=list(range(N_CORES)), trace=trace, **kwargs
    )
    parts = [np.asarray(res.results[c]["out_p"]) for c in range(N_CORES)]
    out = np.stack(
        [
            parts[0] + parts[1] + parts[2] + parts[3],
            parts[4] + parts[5] + parts[6] + parts[7],
        ]
    ).astype(np.float32)
    return out, res


def kernel(**inputs) -> np.ndarray:
    out, _ = _run(inputs, trace=False)
    return out
